# revision 21
# baseline (speedup 1.0000x reference)
"""CAM (channel attention) module kernel for Trainium2, 8-core data-parallel.

Reference computation (per sample b):
    q = conv2d(x, Wq, stride2, 2x2) -> [C, 4096]
    k = conv2d(x, Wk, stride2, 2x2) -> [C, 4096]
    v = conv2d(x, Wv, 1x1)          -> [C, 16384]
    E = q @ k^T                      [C, C]
    att = softmax(rowmax(E) - E)   (== softmin over rows)
    out = att @ v -> [C, H, W]

Kernel strategy (one sample per NeuronCore, B=8 over 8 cores):
  - Precision: the softmax is extremely peaked (energy spans +-200), so
    q/k need ~16 mantissa bits -> split-bf16 conv (3 full-rate bf16
    passes: Wh@xh + Wl@xh + Wh@xl). The hi/lo split of x AND the conv
    weights is done HOST-side: xh+xl bf16 is the same DMA bytes as fp32
    x, and it removes all on-chip split work plus the startup split
    latency chain.
  - Convs are x-stationary producing qT/kT DIRECTLY in [n, c] layout:
    x is shipped in a host-rearranged tap-major layout (stride-2 2x2
    taps partition x exactly, so this is a free "im2col") making every
    stationary chunk a contiguous [cin, 128] slice; the moving operand
    is the conv weights with q and k fused on the free axis
    ([cin, 256] per tap). This kills the PE transposes of the
    W-stationary formulation. 32 chunks of 128 conv outputs; per chunk
    12 bf16 matmuls of 256 rows + 1 fp32 energy matmul accumulating E
    in one PSUM bank. A separate natural-layout bf16 copy of x feeds
    the output matmuls (the strided-AP alternative streams at half
    rate on the PE).
  - softmax via one DVE row-min + one ScalarE exp (bias=rowmin,
    scale=-1) with fused row-sum; 1/Z is folded into att BEFORE the
    M = att @ Wv trick, so out needs no per-row post-scale.
  - out = M @ x in ONE bf16 pass (Mh @ xh): linear path, no exp
    amplification; adds ~2e-3 rel err (gate is 2e-2). The output is
    DMA'd as bf16 (half the drain bytes, ~1.3e-3 more rel err) and
    upcast host-side; copies alternate ScalarE/DVE and the out DMA
    uses the sync+gpsimd queues (ScalarE is busy casting).
  - HAM clock gate: the PE drops to 50% duty after sitting idle, so
    dummy warm-up matmuls run from t~0 through the boot+DMA window and
    keepalive matmuls cover every PE idle slot in the softmax serial
    chain.
  - dma_start costs ~600ns of issue time on the issuing engine, so
    input descriptors are few and ordered by consumption on the sync
    queue (weights, then x chunk 0 quarters, then half-bands, with the
    output-phase tensors last).
"""

import numpy as np

B, C, H, W = 8, 128, 128, 128
HW = H * W             # 16384
N_CORES = 8
NCH = 32               # conv chunks; each covers 512 x-columns -> 128 q rows
WARM_N = 18            # PE warm-up matmuls covering the boot+DMA window

_CACHE = {}


def _build_program(with_qk_bias: bool, with_v_bias: bool):
    import concourse.tile as tile
    from concourse import bacc, mybir
    from concourse.masks import make_identity

    f32 = mybir.dt.float32
    bf16 = mybir.dt.bfloat16
    Ident = mybir.ActivationFunctionType.Identity
    CopyF = mybir.ActivationFunctionType.Copy
    nc = bacc.Bacc(
        "TRN2", target_bir_lowering=False, debug=False, num_devices=N_CORES)

    xrh_d = nc.declare_dram_parameter("xrh", [C, HW], bf16, isOutput=False)
    xrl_d = nc.declare_dram_parameter("xrl", [C, HW], bf16, isOutput=False)
    xnh_d = nc.declare_dram_parameter("xnh", [C, HW], bf16, isOutput=False)
    wqkh_d = nc.declare_dram_parameter("wqkh", [C, 8 * C], bf16, isOutput=False)
    wqkl_d = nc.declare_dram_parameter("wqkl", [C, 8 * C], bf16, isOutput=False)
    wv_d = nc.declare_dram_parameter("wv", [C, C], f32, isOutput=False)
    if with_qk_bias:
        bqk_d = nc.declare_dram_parameter("bqk", [1, 2 * C], f32, isOutput=False)
    if with_v_bias:
        bv_d = nc.declare_dram_parameter("bv", [C, 1], f32, isOutput=False)
    out_d = nc.declare_dram_parameter("out", [C, HW], bf16, isOutput=True)

    with tile.TileContext(nc) as tc:
        with (
            tc.tile_pool(name="const", bufs=1) as const,
            tc.tile_pool(name="xp", bufs=1) as xp,
            tc.tile_pool(name="qkstage", bufs=3) as qkstage,
            tc.tile_pool(name="oout", bufs=6) as oout,
            tc.tile_pool(name="small", bufs=2) as small,
            tc.tile_pool(name="pconv", bufs=2, space="PSUM") as pconv,
            tc.tile_pool(name="psm", bufs=1, space="PSUM") as psm,
            tc.tile_pool(name="pacc", bufs=4, space="PSUM") as pacc,
        ):
            # ---- PE warm-up: keep the tensor engine busy (and p-state
            # ramped) through the fixed boot + initial DMA window so real
            # convs start at full clock. Garbage in, garbage to scratch.
            warm = const.tile([128, 256], bf16, tag="warm")
            nc.gpsimd.memset(warm[:], 0.0)
            for i in range(WARM_N):
                wt = pacc.tile([128, 512], f32, tag="acc", name=f"warm{i}")
                nc.tensor.matmul(wt[:, 0:256], lhsT=warm[:, 0:128], rhs=warm[:],
                                 start=True, stop=True)

            # ---- input DMAs, consumption order, all on the sync queue.
            # First conv chunk needs wqkh + xh cols 0:512 only.
            wqkh_sb = const.tile([C, 8 * C], bf16, tag="wqkh")
            wqkl_sb = const.tile([C, 8 * C], bf16, tag="wqkl")
            # x parts: chunks 0-3 as single-chunk tiles (fine-grained
            # startup), then 1024-col pair tiles.
            xrh_parts, xrl_parts = [], []
            for t, parts in ((0, xrh_parts), (1, xrl_parts)):
                for i in range(4):
                    parts.append(xp.tile([C, 512], bf16, tag=f"x{t}q{i}",
                                         name=f"x{t}q{i}"))
                for i in range(14):
                    parts.append(xp.tile([C, 1024], bf16, tag=f"x{t}b{i}",
                                         name=f"x{t}b{i}"))
            xn_pieces = [xp.tile([C, 4096], bf16, tag=f"xn{j}", name=f"xn{j}")
                         for j in range(4)]

            def x_part(parts, ci):
                if ci < 4:
                    return parts[ci], 0
                return parts[4 + (ci - 4) // 2], 512 * ((ci - 4) % 2)

            nc.sync.dma_start(out=wqkh_sb, in_=wqkh_d[:, :])
            nc.sync.dma_start(out=wqkl_sb, in_=wqkl_d[:, :])
            nc.sync.dma_start(out=xrh_parts[0], in_=xrh_d[:, 0:512])
            nc.sync.dma_start(out=xrl_parts[0], in_=xrl_d[:, 0:512])
            for i in range(1, 4):
                nc.sync.dma_start(out=xrh_parts[i], in_=xrh_d[:, 512 * i:512 * (i + 1)])
                nc.sync.dma_start(out=xrl_parts[i], in_=xrl_d[:, 512 * i:512 * (i + 1)])
            for i in range(14):
                lo, hi = 2048 + 1024 * i, 2048 + 1024 * (i + 1)
                nc.sync.dma_start(out=xrh_parts[4 + i], in_=xrh_d[:, lo:hi])
                nc.sync.dma_start(out=xrl_parts[4 + i], in_=xrl_d[:, lo:hi])
            # wv + natural-layout x at the tail of the sync queue: issued
            # after all conv inputs so they never steal bandwidth from the
            # conv-critical xr feed, yet land well before the output phase.
            wv_sb = const.tile([C, C], f32, tag="wv")
            nc.sync.dma_start(out=wv_sb, in_=wv_d[:, :])
            for j in range(4):
                nc.sync.dma_start(out=xn_pieces[j],
                                  in_=xnh_d[:, j * 4096:(j + 1) * 4096])
            if with_qk_bias:
                bqk_sb = const.tile([1, 2 * C], f32, tag="bqk")
                nc.sync.dma_start(out=bqk_sb, in_=bqk_d[:, :])
                ones1 = const.tile([1, 128], f32, tag="ones1")
                nc.gpsimd.memset(ones1[:], 1.0)
            if with_v_bias:
                bv_sb = const.tile([C, 1], f32, tag="bv")
                nc.sync.dma_start(out=bv_sb, in_=bv_d[:, :])

            ident = const.tile([128, 128], f32, tag="ident")
            make_identity(nc, ident)

            # ---- conv + energy: per chunk, qT|kT [128n, 256] via 12 bf16
            # matmuls (x chunk stationary, fused q|k weights moving), then
            # one fp32 matmul accumulating E. The E matmul for chunk ci-1
            # is emitted during chunk ci so its ScalarE PSUM->SBUF copy has
            # a full chunk of slack.
            E = psm.tile([128, 128], f32, tag="E")
            n_mm = 13 if with_qk_bias else 12
            pend = None
            for ci in range(NCH):
                xh_t, off = x_part(xrh_parts, ci)
                xl_t, _ = x_part(xrl_parts, ci)
                ps = pconv.tile([128, 256], f32, tag="qk")
                # xh terms first (one LDW covers the Wh+Wl pair), xl block
                # last: the lo chunk DMA only gates matmul #9.
                mms = []
                for ab in range(4):
                    xsl = slice(off + ab * 128, off + (ab + 1) * 128)
                    wsl = slice(ab * 256, (ab + 1) * 256)
                    mms.append((xh_t[:, xsl], wqkh_sb[:, wsl]))
                    mms.append((xh_t[:, xsl], wqkl_sb[:, wsl]))
                for ab in range(4):
                    xsl = slice(off + ab * 128, off + (ab + 1) * 128)
                    wsl = slice(ab * 256, (ab + 1) * 256)
                    mms.append((xl_t[:, xsl], wqkh_sb[:, wsl]))
                for idx, (lhsT, rhs) in enumerate(mms):
                    nc.tensor.matmul(ps, lhsT=lhsT, rhs=rhs,
                                     start=(idx == 0), stop=(idx == n_mm - 1))
                if with_qk_bias:
                    nc.tensor.matmul(ps, lhsT=ones1[:, :], rhs=bqk_sb[:, :],
                                     start=False, stop=True,
                                     skip_group_check=True)
                qk_sb = qkstage.tile([128, 256], f32, tag="qks",
                                     name=f"qks{ci}")
                if ci == NCH - 1:
                    nc.scalar.activation(out=qk_sb[:, 0:128], in_=ps[:, 0:128],
                                         func=CopyF, bias=0.0, scale=1.0)
                    nc.vector.tensor_copy(qk_sb[:, 128:256], ps[:, 128:256])
                else:
                    nc.scalar.activation(out=qk_sb, in_=ps, func=CopyF,
                                         bias=0.0, scale=1.0)
                if pend is not None:
                    nc.tensor.matmul(E, lhsT=pend[:, 0:128],
                                     rhs=pend[:, 128:256],
                                     start=(ci == 1), stop=False)
                pend = qk_sb
            for i in range(20, 24):
                wt = pacc.tile([128, 512], f32, tag="acc", name=f"keep{i}")
                nc.tensor.matmul(wt[:, 0:256], lhsT=warm[:, 0:128],
                                 rhs=warm[:], start=True, stop=True)
            nc.tensor.matmul(E, lhsT=pend[:, 0:128], rhs=pend[:, 128:256],
                             start=False, stop=True)

            # ---- softmin over rows: att = exp(rowmin - E) / Z; 1/Z is
            # applied per-partition in the output copies, so the PE path
            # (transpose -> M^T) works on unnormalized exp weights and the
            # reciprocal runs off the critical chain. A few keepalive
            # matmuls hold the PE p-state through the serial window.
            mmin = small.tile([128, 1], f32, tag="mmin")
            nc.vector.tensor_reduce(
                out=mmin, in_=E, axis=mybir.AxisListType.X,
                op=mybir.AluOpType.min)
            w_sb = small.tile([128, 128], f32, tag="w")
            zsum = small.tile([128, 1], f32, tag="z")
            nc.scalar.activation(
                out=w_sb, in_=E, func=mybir.ActivationFunctionType.Exp,
                bias=mmin[:, 0:1], scale=-1.0, accum_out=zsum[:, 0:1])
            rz = small.tile([128, 1], f32, tag="rz")
            nc.vector.reciprocal(rz, zsum)
            att = small.tile([128, 128], f32, tag="att")
            nc.vector.tensor_scalar_mul(att, w_sb, rz[:, 0:1])

            for i in range(12):
                wt = pacc.tile([128, 512], f32, tag="acc", name=f"keep{i}")
                nc.tensor.matmul(wt[:, 0:256], lhsT=warm[:, 0:128],
                                 rhs=warm[:], start=True, stop=True)

            wT_p = psm.tile([128, 128], f32, tag="s2")
            nc.tensor.transpose(wT_p, att, ident)
            wT = small.tile([128, 128], f32, tag="wT")
            nc.vector.tensor_copy(wT, wT_p)

            for i in range(12, 16):
                wt = pacc.tile([128, 512], f32, tag="acc", name=f"keep{i}")
                nc.tensor.matmul(wt[:, 0:256], lhsT=warm[:, 0:128],
                                 rhs=warm[:], start=True, stop=True)

            # M'^T[i, c] = sum_o Wv[o, i] w_exp^T[o, c] (unnormalized);
            # bf16 is enough for the single-pass output matmul.
            MT_p = psm.tile([128, 128], f32, tag="s2")
            nc.tensor.matmul(MT_p, lhsT=wv_sb, rhs=wT, start=True, stop=True)
            Mh = small.tile([128, 128], bf16, tag="Mh")
            nc.vector.tensor_copy(Mh, MT_p)

            if with_v_bias:
                abv_p = psm.tile([128, 1], f32, tag="s2")
                nc.tensor.matmul(abv_p, lhsT=wT, rhs=bv_sb[:, 0:1],
                                 start=True, stop=True)
                abv = small.tile([128, 1], f32, tag="abv")
                nc.vector.tensor_copy(abv, abv_p)

            for i in range(16, 20):
                wt = pacc.tile([128, 512], f32, tag="acc", name=f"keep{i}")
                nc.tensor.matmul(wt[:, 0:256], lhsT=warm[:, 0:128],
                                 rhs=warm[:], start=True, stop=True)

            # ---- out[c, n] = Mh @ xnh, one bf16 pass (1/Z already folded
            # into att -> M); copies alternate ScalarE/DVE and the out DMA
            # rotates the three queues at half-band granularity.
            # sync + gpsimd only: ScalarE is busy casting o_band and a
            # ~600ns dma-issue on it would make it the band straggler.
            out_dma_engines = [nc.sync, nc.gpsimd]
            qe = 0
            for j in range(8):
                o_band = oout.tile([128, 2048], bf16, tag="oband")
                for s in range(4):
                    ci = j * 4 + s
                    o_ps = pacc.tile([128, 512], f32, tag="acc",
                                     name=f"ops{ci}")
                    xn_t = xn_pieces[j // 2]
                    xoff = (j % 2) * 2048 + s * 512
                    nc.tensor.matmul(o_ps, lhsT=Mh[:, :],
                                     rhs=xn_t[:, xoff:xoff + 512],
                                     start=True, stop=True)
                    dst = o_band[:, s * 512:(s + 1) * 512]
                    if with_v_bias:
                        if s % 2 == 0:
                            nc.scalar.activation(out=dst, in_=o_ps, func=Ident,
                                                 bias=abv[:, 0:1], scale=1.0)
                        else:
                            nc.vector.tensor_scalar_add(dst, o_ps, abv[:, 0:1])
                    else:
                        if s % 2 == 0:
                            nc.scalar.activation(out=dst, in_=o_ps, func=CopyF,
                                                 bias=0.0, scale=1.0)
                        else:
                            nc.vector.tensor_copy(dst, o_ps)
                pieces = 4 if j == 7 else 2
                psz = 2048 // pieces
                for h in range(pieces):
                    off = j * 2048 + h * psz
                    out_dma_engines[qe % 2].dma_start(
                        out=out_d[:, off:off + psz],
                        in_=o_band[:, h * psz:(h + 1) * psz])
                    qe += 1

    nc.compile()
    return nc


def kernel(x, Wq, bq, Wk, bk, Wv, bv):
    import ml_dtypes
    from concourse.bass_utils import run_bass_kernel_spmd

    bf16 = ml_dtypes.bfloat16
    x = np.ascontiguousarray(np.asarray(x, dtype=np.float32))
    Wq = np.asarray(Wq, dtype=np.float32)
    Wk = np.asarray(Wk, dtype=np.float32)
    Wv = np.asarray(Wv, dtype=np.float32)
    bq = np.asarray(bq, dtype=np.float32)
    bk = np.asarray(bk, dtype=np.float32)
    bv = np.asarray(bv, dtype=np.float32)

    with_qk_bias = bool(np.any(bq) or np.any(bk))
    with_v_bias = bool(np.any(bv))

    key = (with_qk_bias, with_v_bias)
    if key not in _CACHE:
        _CACHE[key] = _build_program(with_qk_bias, with_v_bias)
    nc = _CACHE[key]

    # weight layout: wcat[cin, ab*256 + {0:128 -> q, 128:256 -> k} cout]
    wq_t = Wq.transpose(1, 2, 3, 0).reshape(C, 4, 1, C)
    wk_t = Wk.transpose(1, 2, 3, 0).reshape(C, 4, 1, C)
    wcat = np.concatenate([wq_t, wk_t], axis=2).reshape(C, 8 * C)
    wqkh = wcat.astype(bf16)
    wqkl = (wcat - wqkh.astype(np.float32)).astype(bf16)
    wqkh = np.ascontiguousarray(wqkh)
    wqkl = np.ascontiguousarray(wqkl)
    wv = np.ascontiguousarray(Wv.reshape(C, C))

    # conv layout: xr2[c, ci, a, b, di, w] = x[c, 4ci+2di+a, 2w+b]
    # (stride-2 2x2 taps partition x exactly; host-side "im2col")
    xr2 = x.reshape(B, C, 32, 2, 2, 64, 2).transpose(
        0, 1, 2, 4, 6, 3, 5).reshape(B, C, HW)
    xrh = xr2.astype(bf16)
    xrl = (xr2 - xrh.astype(np.float32)).astype(bf16)
    xnh = x.reshape(B, C, HW).astype(bf16)

    in_maps = []
    for b in range(B):
        m = {
            "xrh": np.ascontiguousarray(xrh[b]),
            "xrl": np.ascontiguousarray(xrl[b]),
            "xnh": np.ascontiguousarray(xnh[b]),
            "wqkh": wqkh,
            "wqkl": wqkl,
            "wv": wv,
        }
        if with_qk_bias:
            m["bqk"] = np.ascontiguousarray(
                np.concatenate([bq, bk]).reshape(1, 2 * C))
        if with_v_bias:
            m["bv"] = np.ascontiguousarray(bv.reshape(C, 1))
        in_maps.append(m)

    res = run_bass_kernel_spmd(nc, in_maps, list(range(N_CORES)))
    # output rides DMA as bf16 (half the drain bytes; ~2e-3 extra rel err
    # against the 2e-2 gate) and is upcast host-side.
    out = np.stack([np.asarray(res.results[i]["out"]) for i in range(N_CORES)])
    return out.reshape(B, C, H, W).astype(np.float32)
